# revision 2
# baseline (speedup 1.0000x reference)
"""Trainium2 Bass kernel for nn_CeptaContextBlock (B=4, T=4096, D=1024, P=512, ALPHA=4, PR=64).

Math (after algebraic simplification of the reference):
    W_comb = W_toP + sum_a W_U[:,:,a] * W_V[:,a]          (host precompute)
    t    = x @ W_comb                                     (B,T,P)
    Fg   = sigmoid(x @ W_F)                               (B,T,P)
    lam  = sigmoid(Fg @ W_lam)                            (B,T,PR)
    u    = t @ B_mat                                      (B,T,PR)
    s    = scan: s_i = lam_i * s_{i-1} + u_i along T      (B,T,PR)
    h    = (t + s @ C_mat) @ W_fromP                      (B,T,D)

Sharding: 8 cores; core c handles batch b=c//2, token half c%2 (2048 tokens).
The scan carry across each (even, odd) core pair is exchanged with a tiny
AllGather of the final scan state; odd cores use it as the scan initial value
(masked via a per-core 0/1 input so the SPMD program has no branches).
"""

import os
import sys

import numpy as np

for _p in ("/opt/trn_rl_repo", "/root/.axon_site/_ro/trn_rl_repo"):
    if os.path.isdir(_p) and _p not in sys.path:
        sys.path.append(_p)

import ml_dtypes

import concourse.bass as bass
import concourse.bacc as bacc
import concourse.mybir as mybir
import concourse.tile as tile
from concourse import bass_utils

B, T, D, P, ALPHA, PR = 4, 4096, 1024, 512, 4, 64
NCORES = 8
TL = T // 2          # tokens per core
KD = D // 128        # 8 d-chunks (contraction for big matmul)
PT = P // 128        # 4 p-tiles
CH = 512             # token chunk (free dim per matmul)
NCH = TL // CH       # 4 token chunks per core
F32 = mybir.dt.float32
BF16 = mybir.dt.bfloat16
SIG = mybir.ActivationFunctionType.Sigmoid
MUL = mybir.AluOpType.mult
ADD = mybir.AluOpType.add

_CACHE = {}


def build_program(ncores: int = NCORES):
    """Build the SPMD Tile program (same NEFF on all cores)."""
    nc = bacc.Bacc(
        "TRN2", target_bir_lowering=False, debug=False, num_devices=ncores
    )

    xt_d = nc.dram_tensor("xt", [D, TL], BF16, kind="ExternalInput")
    wcf_d = nc.dram_tensor("wcf", [D, 2 * P], BF16, kind="ExternalInput")
    wlam_d = nc.dram_tensor("wlam", [P, PR], BF16, kind="ExternalInput")
    bmat_d = nc.dram_tensor("bmat", [P, PR], BF16, kind="ExternalInput")
    cmat_d = nc.dram_tensor("cmat", [PR, P], BF16, kind="ExternalInput")
    wfp_d = nc.dram_tensor("wfp", [P, D], BF16, kind="ExternalInput")
    cmask_d = nc.dram_tensor("cmask", [PR, 1], F32, kind="ExternalInput")
    h_d = nc.dram_tensor("h", [TL, D], BF16, kind="ExternalOutput")

    with tile.TileContext(nc) as tc:
        with (
            tc.tile_pool(name="wp", bufs=1) as wp,
            tc.tile_pool(name="xp", bufs=16) as xp,
            tc.tile_pool(name="big", bufs=1) as big,
            tc.tile_pool(name="hp", bufs=3) as hp,
            tc.tile_pool(name="ppa", bufs=3, space="PSUM") as ppa,
            tc.tile_pool(name="pps", bufs=2, space="PSUM") as pps,
            tc.tile_pool(name="pph", bufs=3, space="PSUM") as pph,
            tc.tile_pool(name="dram", bufs=1, space="DRAM") as dp,
        ):
            # ---- weights to SBUF ----
            wcf_sb = []
            for k in range(KD):
                w = wp.tile([128, 2 * P], BF16, tag=f"wcf{k}", name=f"wcf{k}")
                nc.sync.dma_start(w[:], wcf_d[k * 128 : (k + 1) * 128, :])
                wcf_sb.append(w)
            wfp_sb = []
            for k in range(PT):
                w = wp.tile([128, D], BF16, tag=f"wfp{k}", name=f"wfp{k}")
                nc.sync.dma_start(w[:], wfp_d[k * 128 : (k + 1) * 128, :])
                wfp_sb.append(w)
            wlam_sb, bmat_sb = [], []
            for k in range(PT):
                w = wp.tile([128, PR], BF16, tag=f"wlam{k}", name=f"wlam{k}")
                nc.sync.dma_start(w[:], wlam_d[k * 128 : (k + 1) * 128, :])
                wlam_sb.append(w)
                w = wp.tile([128, PR], BF16, tag=f"bmat{k}", name=f"bmat{k}")
                nc.sync.dma_start(w[:], bmat_d[k * 128 : (k + 1) * 128, :])
                bmat_sb.append(w)
            cmat_sb = wp.tile([PR, P], BF16, tag="cmat", name="cmat")
            nc.sync.dma_start(cmat_sb[:], cmat_d[:, :])
            cmask_sb = wp.tile([PR, 1], F32, tag="cmask", name="cmask")
            nc.sync.dma_start(cmask_sb[:], cmask_d[:, :])

            # ---- persistent activations ----
            t_sb = [
                big.tile([128, TL], BF16, tag=f"t{m}", name=f"t{m}")
                for m in range(PT)
            ]
            fg_sb = [
                big.tile([128, TL], BF16, tag=f"fg{m}", name=f"fg{m}")
                for m in range(PT)
            ]
            ttil_sb = [
                big.tile([128, TL], BF16, tag=f"ttil{m}", name=f"ttil{m}")
                for m in range(PT)
            ]
            lam_sb = big.tile([PR, TL], F32, tag="lam", name="lam")
            u_sb = big.tile([PR, TL], F32, tag="u", name="u")
            s1_sb = big.tile([PR, TL], F32, tag="s1", name="s1")
            s2_sb = big.tile([PR, TL], BF16, tag="s2", name="s2")
            ceff_sb = big.tile([PR, 1], F32, tag="ceff", name="ceff")
            carry_sb = big.tile([PR, 1], F32, tag="carry", name="carry")

            # ---- stage 1: big matmuls (t | Fg), then lam/u, per token chunk ----
            for c in range(NCH):
                cs = slice(c * CH, (c + 1) * CH)
                xt_t = []
                for k in range(KD):
                    xtile = xp.tile([128, CH], BF16, tag="xt", name=f"xt{c}_{k}")
                    nc.sync.dma_start(xtile[:], xt_d[k * 128 : (k + 1) * 128, cs])
                    xt_t.append(xtile)
                for m in range(2 * PT):
                    pa = ppa.tile([128, CH], F32, tag="pa", name=f"pa{c}_{m}")
                    for k in range(KD):
                        nc.tensor.matmul(
                            pa[:],
                            wcf_sb[k][:, m * 128 : (m + 1) * 128],
                            xt_t[k][:],
                            start=(k == 0),
                            stop=(k == KD - 1),
                        )
                    if m < PT:
                        nc.vector.tensor_copy(t_sb[m][:, cs], pa[:])
                    else:
                        nc.scalar.activation(fg_sb[m - PT][:, cs], pa[:], SIG)
                # lam / u for this chunk
                pl = pps.tile([PR, CH], F32, tag="ps", name=f"pl{c}")
                for k in range(PT):
                    nc.tensor.matmul(
                        pl[:],
                        wlam_sb[k][:],
                        fg_sb[k][:, cs],
                        start=(k == 0),
                        stop=(k == PT - 1),
                    )
                nc.scalar.activation(lam_sb[:, cs], pl[:], SIG)
                pu = pps.tile([PR, CH], F32, tag="ps", name=f"pu{c}")
                for k in range(PT):
                    nc.tensor.matmul(
                        pu[:],
                        bmat_sb[k][:],
                        t_sb[k][:, cs],
                        start=(k == 0),
                        stop=(k == PT - 1),
                    )
                nc.vector.tensor_copy(u_sb[:, cs], pu[:])

            # ---- stage 2: local scan, carry exchange, final scan ----
            nc.vector.tensor_tensor_scan(
                s1_sb[:], lam_sb[:], u_sb[:], 0.0, op0=MUL, op1=ADD
            )
            cin_bounce = dp.tile([PR, 1], F32, name="cin_bounce")
            cout_bounce = dp.tile([2 * PR, 1], F32, name="cout_bounce")
            nc.sync.dma_start(cin_bounce[:], s1_sb[:, TL - 1 : TL])
            nc.gpsimd.collective_compute(
                "AllGather",
                mybir.AluOpType.bypass,
                replica_groups=[[0, 1], [2, 3], [4, 5], [6, 7]],
                ins=[cin_bounce.opt()],
                outs=[cout_bounce.opt()],
            )
            nc.sync.dma_start(carry_sb[:], cout_bounce[0:PR, :])
            nc.vector.tensor_mul(ceff_sb[:], carry_sb[:], cmask_sb[:])
            nc.vector.tensor_tensor_scan(
                s2_sb[:], lam_sb[:], u_sb[:], ceff_sb[:], op0=MUL, op1=ADD
            )

            # ---- stage 3: t_tilde = t + s2 @ C, then h = t_tilde @ W_fromP ----
            for c in range(NCH):
                cs = slice(c * CH, (c + 1) * CH)
                for m in range(PT):
                    pt = pps.tile([128, CH], F32, tag="ps", name=f"pt{c}_{m}")
                    nc.tensor.matmul(
                        pt[:],
                        cmat_sb[:, m * 128 : (m + 1) * 128],
                        s2_sb[:, cs],
                        start=True,
                        stop=True,
                    )
                    nc.vector.tensor_add(ttil_sb[m][:, cs], t_sb[m][:, cs], pt[:])
                for tt in range(4 * c, 4 * (c + 1)):
                    ts_ = slice(tt * 128, (tt + 1) * 128)
                    h_t = hp.tile([128, D], BF16, tag="hs", name=f"h{tt}")
                    for dc in range(2):
                        ph = pph.tile([128, CH], F32, tag="ph", name=f"ph{tt}_{dc}")
                        for k in range(PT):
                            nc.tensor.matmul(
                                ph[:],
                                ttil_sb[k][:, ts_],
                                wfp_sb[k][:, dc * CH : (dc + 1) * CH],
                                start=(k == 0),
                                stop=(k == PT - 1),
                            )
                        nc.scalar.activation(
                            h_t[:, dc * CH : (dc + 1) * CH],
                            ph[:],
                            mybir.ActivationFunctionType.Copy,
                        )
                    nc.sync.dma_start(h_d[ts_, :], h_t[:])

    nc.compile()
    return nc


def _prep_inputs(x, W_toP, W_U, W_F, W_V, W_lam, B_mat, C_mat, W_fromP):
    """Host-side sharding prep: weight fold, bf16 cast, per-core x transpose."""
    bf = ml_dtypes.bfloat16
    W_comb = W_toP + (W_U * W_V[None, :, :]).sum(-1)
    wcf = np.concatenate([W_comb, W_F], axis=1).astype(bf)
    wlam = np.asarray(W_lam, np.float32).astype(bf)
    bmat = np.asarray(B_mat, np.float32).astype(bf)
    cmat = np.asarray(C_mat, np.float32).astype(bf)
    wfp = np.asarray(W_fromP, np.float32).astype(bf)
    in_maps = []
    for c in range(NCORES):
        b, half = c // 2, c % 2
        xs = np.ascontiguousarray(
            np.asarray(x[b, half * TL : (half + 1) * TL, :], np.float32).T
        ).astype(bf)
        cmask = np.full((PR, 1), float(half), np.float32)
        in_maps.append(
            {
                "xt": xs,
                "wcf": wcf,
                "wlam": wlam,
                "bmat": bmat,
                "cmat": cmat,
                "wfp": wfp,
                "cmask": cmask,
            }
        )
    return in_maps


def kernel(**inputs) -> np.ndarray:
    if "nc" not in _CACHE:
        _CACHE["nc"] = build_program()
    nc = _CACHE["nc"]
    in_maps = _prep_inputs(**inputs)
    trace = bool(int(os.environ.get("CEPTA_TRACE", "0")))
    res = bass_utils.run_bass_kernel_spmd(
        nc,
        in_maps,
        core_ids=list(range(NCORES)),
        trace=trace,
        trace_cores=[0] if trace else None,
    )
    _CACHE["last_result"] = res
    out = np.empty((B, T, D), np.float32)
    for c in range(NCORES):
        b, half = c // 2, c % 2
        out[b, half * TL : (half + 1) * TL, :] = res.results[c]["h"].astype(
            np.float32
        )
    return out
